# revision 48
# baseline (speedup 1.0000x reference)
"""GCN (2-layer, GCNConv + log_softmax) on 8 Trainium2 NeuronCores.

Strategy (1D node partition):
  - Nodes padded to N_PAD = 392*128, sharded contiguously: 49 blocks of 128
    dst-nodes per core.
  - CPU preprocessing: add self-loops, compute dinv=1/sqrt(deg), sort edges by
    dst, pack per (core, block) into edge tiles of 128.  dinv[src] is folded
    into the gathered feature tables (h, h2 stored pre-scaled), dinv[dst] is
    applied after each aggregation, so the selector is a pure one-hot
    (single is_equal on DVE; padding slots get seg=255).
  - Edge->feature gathers use the hardware gather instruction
    (gpsimd.dma_gather / InstDMAGatherAnt, single_packet=False, descriptor
    generation spread over 4 SWDGE queues): amortizes the SWDGE cost over
    GB blocks of edges (the per-tile indirect-DMA variant paid ~1us fixed
    cost per 128 edges and dominated the runtime; multi-column offset APs
    on indirect_dma_start silently misgather on HW).
  - Gather indices are int16, so each layer's feature table is SPLIT in two:
    a "lo" table (per-core rows [0, 4096) -> exactly 32768 rows) and a "hi"
    table (rows [4096, 6272)).  Each table is AllGathered separately, and
    the lo collective is issued as soon as its rows are stored -- overlapping
    the bulk of the collective with the remaining blocks' compute (a single
    Shared tensor may only have one writer, so chunking one table is not
    possible).  Edges are packed into lo tiles then hi tiles per block.
  - On device per core:
      GEMM1: h = (x_shard @ W1) * dinv  (PE bf16, streamed x, ACT-scaled copy)
      AllGather h_lo early, h_hi at phase end -> bf16 h tables [*, 128]
      Agg1 per dst block: lo+hi gathers, one-hot selector, segment-sum via PE
        matmul accumulation into PSUM [hid, dst]; * dinv[dst] (DVE), relu+b1
        (ACT), fused GEMM2 (bf16) -> h2 = (a1 @ W2) * dinv stored bf16
        128-padded (gather rows must be a multiple of 256 bytes).
      AllGather h2_lo early, h2_hi at phase end
      Agg2 per dst block: lo+hi gathers of h2[src], bf16 selector, accumulate
        [dst, 128]; * dinv[dst] + b2 on the first 40 cols, log_softmax; store.
  - Host concatenates the 8 output shards and strips padding.
"""

import math

import numpy as np
import ml_dtypes

P = 128
NCORES = 8
LO_ROWS = 4096          # per-core rows in the "lo" table (8*4096 = 32768,
                        # the int16 gather index limit)

# Full-problem constants (hardcoded per harness contract).
N_NODES = 50000
N_EDGES = 800000
F_IN = 512
HIDDEN = 128
N_CLASSES = 40
NCLS_PAD = 128          # h2 table width: 128 * 2B = 256B rows (bf16)

# Runtime-tunable knobs (test.py may override before calling kernel()).
TRACE = False
TRACE_KWARGS = {}
CH = 1          # AllGather chunks (1 = single collective per layer)
SG = 7          # blocks per store DMA (must divide blocks-per-AG-chunk)
GB = 5          # dst blocks per gather pair (amortizes Q7 gather launch)

LAST_RESULT = {}        # test.py introspection (exec time etc.)


# --------------------------------------------------------------------------
# CPU preprocessing
# --------------------------------------------------------------------------

def _balance_nodes(deg_w, nblk):
    """LPT bin packing: nodes -> blocks of 128, balancing in-edge counts.

    Returns pad_slot[v] = global padded slot (block*128 + row-in-block).
    """
    import heapq

    order_nodes = np.argsort(-deg_w, kind="stable")
    heap = [(0, b) for b in range(nblk)]
    heapq.heapify(heap)
    fill = np.zeros(nblk, np.int64)
    pad_slot = np.empty(len(deg_w), np.int64)
    for v in order_nodes:
        while True:
            load, b = heapq.heappop(heap)
            if fill[b] < P:
                break
        pad_slot[v] = b * P + fill[b]
        fill[b] += 1
        heapq.heappush(heap, (load + int(deg_w[v]), b))
    return pad_slot


def _preprocess(edge_index, n_nodes, blocks_per_core, ch):
    """Sort edges (plus self-loops) by dst; pack per block into lo/hi tiles.

    Returns (idxlo, idxhi, segs, dinv_pad, T_lo, T_hi, perm):
      idxlo [NCORES, 128, BPC*T_lo*8]  int16  gather indices, 16-wrapped
      idxhi [NCORES, 128, BPC*T_hi*8]  int16  (table-local)
      segs  [NCORES, 128, BPC*(T_lo+T_hi)] f32  dst%128 (255 for padding)
      dinv_pad [n_pad] f32
      perm  [n_nodes]  padded global row of each original node
    """
    shard = blocks_per_core * P
    n_pad = NCORES * shard
    qsz = shard // ch
    nblk = NCORES * blocks_per_core
    src = np.asarray(edge_index[0], dtype=np.int64)
    dst = np.asarray(edge_index[1], dtype=np.int64)

    deg = np.bincount(dst, minlength=n_nodes).astype(np.float32) + 1.0
    dinv = (1.0 / np.sqrt(deg)).astype(np.float32)

    # balance per-block edge counts so the uniform tile counts hug the mean
    pad_slot = _balance_nodes(deg.astype(np.int64), nblk)
    blk_of = pad_slot // P
    perm = (blk_of // blocks_per_core) * shard + \
        (blk_of % blocks_per_core) * P + pad_slot % P

    dinv_pad = np.zeros(n_pad, np.float32)
    dinv_pad[perm] = dinv

    loops = np.arange(n_nodes, dtype=np.int64)
    all_src = perm[np.concatenate([src, loops])]
    all_dst = perm[np.concatenate([dst, loops])]

    order = np.argsort(all_dst, kind="stable")
    s_src = all_src[order]
    s_dst = all_dst[order]

    # Two gather tables, each AllGathered separately so the "lo" collective
    # (per-core rows [0, 4096) -> exactly 32768 table rows) can be issued
    # before the phase finishes and overlap with the remaining compute.
    # lo table row of node v = core*4096 + r   (r = local row < 4096)
    # hi table row of node v = core*2176 + (r - 4096)
    c_of = s_src // shard
    r0 = s_src % shard
    is_lo = r0 < LO_ROWS
    s_row = np.where(
        is_lo,
        c_of * LO_ROWS + r0,
        c_of * (shard - LO_ROWS) + (r0 - LO_ROWS),
    ).astype(np.int64)

    blk = s_dst // P

    # per-block lo/hi counts -> uniform tile counts
    cnt_lo = np.bincount(blk[is_lo], minlength=nblk)
    cnt_hi = np.bincount(blk[~is_lo], minlength=nblk)
    T_lo = max(1, int(math.ceil(cnt_lo.max() / P)))
    T_hi = max(1, int(math.ceil(cnt_hi.max() / P)))
    T = T_lo + T_hi

    segs = np.full((NCORES, P, blocks_per_core * T), 255.0, np.float32)
    idxlo16 = np.zeros((NCORES, 16, blocks_per_core * T_lo * 8), np.int16)
    idxhi16 = np.zeros((NCORES, 16, blocks_per_core * T_hi * 8), np.int16)

    for b in range(nblk):
        c, bl = divmod(b, blocks_per_core)
        in_b = blk == b
        for lo_flag, idx_arr, T_part, t_base, w in (
            (True, idxlo16, T_lo, 0, T_lo * 8),
            (False, idxhi16, T_hi, T_lo, T_hi * 8),
        ):
            m = in_b & (is_lo == lo_flag)
            rows = s_row[m]          # already table-local for both tables
            dloc = s_dst[m] % P
            s = np.arange(len(rows))
            # seg for slot s -> tile t_base + s//128, partition s%128
            segs[c, s % P, bl * T + t_base + s // P] = dloc
            # gather idx, 16-wrapped: slot s -> [s%16, s//16]
            idx_arr[c, s % 16, bl * w + s // 16] = rows.astype(np.int16)
    # replicate the 16-partition wrap across all 8 GPSIMD core stripes
    idxlo = np.tile(idxlo16, (1, 8, 1))
    idxhi = np.tile(idxhi16, (1, 8, 1))
    return idxlo, idxhi, segs, dinv_pad, T_lo, T_hi, perm


# --------------------------------------------------------------------------
# Device program
# --------------------------------------------------------------------------

def _build_program(f_in, hidden, blocks_per_core, T_lo, T_hi, ch, sg, gb):
    import concourse.bacc as bacc
    import concourse.mybir as mybir
    import concourse.tile as tile

    dt = mybir.dt
    bf16 = dt.bfloat16
    f32 = dt.float32

    T = T_lo + T_hi
    ncp = NCLS_PAD
    shard = blocks_per_core * P
    n_pad = NCORES * shard
    kt = f_in // P
    qb = blocks_per_core // ch       # blocks per AG chunk
    qsz = qb * P
    wlo, whi = T_lo * 8, T_hi * 8    # idx cols per block
    assert blocks_per_core % ch == 0 and qb % sg == 0

    nc = bacc.Bacc(
        "TRN2",
        target_bir_lowering=False,
        debug=False,
        enable_asserts=False,
        num_devices=NCORES,
        num_swdge_queues=4,
    )

    xt_d = nc.dram_tensor("xt", [P, blocks_per_core * kt * P], bf16,
                          kind="ExternalInput")
    w1_d = nc.dram_tensor("w1", [P, kt * hidden], bf16, kind="ExternalInput")
    b1_d = nc.dram_tensor("b1", [P, 1], f32, kind="ExternalInput")
    w2_d = nc.dram_tensor("w2", [hidden, ncp], bf16, kind="ExternalInput")
    b2_d = nc.dram_tensor("b2t", [P, N_CLASSES], f32, kind="ExternalInput")
    iota_d = nc.dram_tensor("iotaw", [P, T * P], bf16, kind="ExternalInput")
    ilo_d = nc.dram_tensor("idxlo", [P, blocks_per_core * wlo], dt.int16,
                           kind="ExternalInput")
    ihi_d = nc.dram_tensor("idxhi", [P, blocks_per_core * whi], dt.int16,
                           kind="ExternalInput")
    segs_d = nc.dram_tensor("segs", [P, blocks_per_core * T], bf16,
                            kind="ExternalInput")
    dinvp_d = nc.dram_tensor("dinvp", [P, blocks_per_core], f32,
                             kind="ExternalInput")
    dinvf_d = nc.dram_tensor("dinvf", [P, shard], f32, kind="ExternalInput")
    out_d = nc.dram_tensor("out", [shard, N_CLASSES], f32,
                           kind="ExternalOutput")

    RG = [list(range(NCORES))]
    Relu = mybir.ActivationFunctionType.Relu
    Copy = mybir.ActivationFunctionType.Copy
    Exp = mybir.ActivationFunctionType.Exp
    Ln = mybir.ActivationFunctionType.Ln

    with tile.TileContext(nc) as tc:
        with (
            tc.tile_pool(name="const", bufs=1) as const,
            tc.tile_pool(name="dram", bufs=1, space="DRAM") as dram,
            tc.tile_pool(name="sb", bufs=3) as sb,
            tc.tile_pool(name="psum", bufs=2, space="PSUM") as psum,
        ):
            lo, hi = LO_ROWS, shard - LO_ROWS
            h_ag_in = dram.tile([shard, hidden], bf16)
            h_lo = dram.tile([NCORES * lo, hidden], bf16, addr_space="Shared")
            h_hi = dram.tile([NCORES * hi, hidden], bf16, addr_space="Shared")
            h2_ag_in = dram.tile([shard, ncp], bf16)
            h2_lo = dram.tile([NCORES * lo, ncp], bf16, addr_space="Shared")
            h2_hi = dram.tile([NCORES * hi, ncp], bf16, addr_space="Shared")

            w1_sb = const.tile([P, kt * hidden], bf16)
            nc.sync.dma_start(out=w1_sb[:], in_=w1_d[:])
            b1_sb = const.tile([P, 1], f32)
            nc.sync.dma_start(out=b1_sb[:], in_=b1_d[:])
            w2_sb = const.tile([hidden, ncp], bf16)
            nc.sync.dma_start(out=w2_sb[:], in_=w2_d[:])
            b2_sb = const.tile([P, N_CLASSES], f32)
            nc.sync.dma_start(out=b2_sb[:], in_=b2_d[:])
            iota_sb = const.tile([P, T * P], bf16)
            nc.sync.dma_start(out=iota_sb[:], in_=iota_d[:])
            ilo_sb = const.tile([P, blocks_per_core * wlo], dt.int16)
            nc.sync.dma_start(out=ilo_sb[:], in_=ilo_d[:])
            ihi_sb = const.tile([P, blocks_per_core * whi], dt.int16)
            nc.sync.dma_start(out=ihi_sb[:], in_=ihi_d[:])
            segs_sb = const.tile([P, blocks_per_core * T], bf16)
            nc.sync.dma_start(out=segs_sb[:], in_=segs_d[:])
            dinvp_sb = const.tile([P, blocks_per_core], f32)
            nc.sync.dma_start(out=dinvp_sb[:], in_=dinvp_d[:])
            dinvf_sb = const.tile([P, shard], f32)
            nc.sync.dma_start(out=dinvf_sb[:], in_=dinvf_d[:])

            iota3 = iota_sb[:].rearrange("p (t d) -> p t d", d=P)

            def build_selector(b, seldt, tag):
                g0 = b * T
                sel = sb.tile([P, T * P], seldt, tag=tag)
                nc.vector.tensor_tensor(
                    out=sel[:].rearrange("p (t d) -> p t d", d=P),
                    in0=iota3,
                    in1=segs_sb[:, g0:g0 + T].to_broadcast([P, T, P]),
                    op=mybir.AluOpType.is_equal,
                )
                return sel

            gq = [0]  # rotating SWDGE queue assignment

            def gather_group(b0, gsz, t_lo, t_hi, F, tag):
                """One lo + one hi gather covering blocks [b0, b0+gsz)."""
                msg = sb.tile([P, gb * T * F], bf16, tag=tag, bufs=2)
                nc.gpsimd.dma_gather(
                    out_ap=msg[:, :gsz * T_lo * F].rearrange(
                        "p (t f) -> p t f", f=F),
                    in_ap=t_lo[:],
                    idxs_ap=ilo_sb[:, b0 * wlo:(b0 + gsz) * wlo],
                    num_idxs=gsz * T_lo * P,
                    num_idxs_reg=gsz * T_lo * P,
                    elem_size=F,
                    single_packet=False,
                    queue_num=gq[0],
                )
                gq[0] = (gq[0] + 1) % 4
                nc.gpsimd.dma_gather(
                    out_ap=msg[:, gsz * T_lo * F:gsz * T * F].rearrange(
                        "p (t f) -> p t f", f=F),
                    in_ap=t_hi[:],
                    idxs_ap=ihi_sb[:, b0 * whi:(b0 + gsz) * whi],
                    num_idxs=gsz * T_hi * P,
                    num_idxs_reg=gsz * T_hi * P,
                    elem_size=F,
                    single_packet=False,
                    queue_num=gq[0],
                )
                gq[0] = (gq[0] + 1) % 4
                return msg

            def ag_pair(src_dram, out_lo, out_hi, which):
                """Issue the lo or hi AllGather of a phase's table."""
                if which == "lo":
                    ins, outs = src_dram[0:lo, :], out_lo[:]
                else:
                    ins, outs = src_dram[lo:shard, :], out_hi[:]
                nc.gpsimd.collective_compute(
                    "AllGather",
                    mybir.AluOpType.bypass,
                    replica_groups=RG,
                    ins=[ins],
                    outs=[outs],
                )

            def tile_off(k, gsz, t):
                """msg tile index of block k's selector tile t in a group."""
                if t < T_lo:
                    return k * T_lo + t
                return gsz * T_lo + k * T_hi + (t - T_lo)

            # ---------------- Phase 1: GEMM1 (h = (x @ W1) * dinv) --------
            for q in range(ch):
                h_acc = None
                xt_t = None
                for j in range(qb):
                    i = q * qb + j
                    jj = j % sg
                    if jj == 0:
                        xt_t = sb.tile([P, sg * kt * P], bf16, tag="xt_t",
                                       bufs=2)
                        nc.sync.dma_start(
                            out=xt_t[:],
                            in_=xt_d[:, i * kt * P:(i + sg) * kt * P],
                        )
                    ps = psum.tile([P, P], f32, tag="psA")
                    for k in range(kt):
                        c0 = (jj * kt + k) * P
                        nc.tensor.matmul(
                            out=ps[:],
                            lhsT=xt_t[:, c0:c0 + P],
                            rhs=w1_sb[:, k * hidden:(k + 1) * hidden],
                            start=(k == 0),
                            stop=(k == kt - 1),
                        )
                    if jj == 0:
                        h_acc = sb.tile([P, sg * hidden], bf16, tag="h_acc",
                                        bufs=2)
                    nc.scalar.activation(
                        out=h_acc[:, jj * hidden:(jj + 1) * hidden],
                        in_=ps[:], func=Copy, scale=dinvp_sb[:, i:i + 1],
                    )
                    if jj == sg - 1:
                        b0 = i - sg + 1
                        nc.sync.dma_start(
                            out=h_ag_in[b0 * P:(i + 1) * P, :].rearrange(
                                "(g p) h -> p g h", p=P),
                            in_=h_acc[:].rearrange("p (g h) -> p g h",
                                                   h=hidden),
                        )
                        if b0 * P < lo <= (i + 1) * P:
                            # lo-table rows complete: start its AllGather
                            # while the remaining blocks compute.
                            ag_pair(h_ag_in, h_lo, h_hi, "lo")
                ag_pair(h_ag_in, h_lo, h_hi, "hi")

            # ------- Phase 2: Agg1 * dinv, relu + b1, GEMM2, * dinv -------
            for q in range(ch):
                h2_acc = None
                j = 0
                while j < qb:
                    gsz = min(gb, qb - j)
                    bg = q * qb + j
                    msg = gather_group(bg, gsz, h_lo, h_hi, hidden, "msg")
                    for kk in range(gsz):
                        b = bg + kk
                        sel = build_selector(b, bf16, "sel")
                        ps1 = psum.tile([P, P], f32, tag="psA")
                        for t in range(T):
                            mo = tile_off(kk, gsz, t) * hidden
                            nc.tensor.matmul(
                                out=ps1[:],
                                lhsT=msg[:, mo:mo + hidden],
                                rhs=sel[:, t * P:(t + 1) * P],
                                start=(t == 0),
                                stop=(t == T - 1),
                            )
                        # ps1 = agg1^T : [hid, dst]; * dinv[dst], relu(.+b1)
                        t1 = sb.tile([P, P], f32, tag="t1")
                        nc.vector.tensor_tensor(
                            out=t1[:], in0=ps1[:],
                            in1=dinvf_sb[:, b * P:(b + 1) * P],
                            op=mybir.AluOpType.mult,
                        )
                        a1 = sb.tile([P, P], bf16, tag="a1")
                        nc.scalar.activation(
                            out=a1[:], in_=t1[:], func=Relu,
                            bias=b1_sb[:, 0:1],
                        )
                        ps2 = psum.tile([P, ncp], f32, tag="psB")
                        nc.tensor.matmul(
                            out=ps2[:], lhsT=a1[:], rhs=w2_sb[:],
                            start=True, stop=True,
                        )
                        jj = (j + kk) % sg
                        if jj == 0:
                            h2_acc = sb.tile([P, sg * ncp], bf16,
                                             tag="h2_acc", bufs=2)
                        nc.scalar.activation(
                            out=h2_acc[:, jj * ncp:(jj + 1) * ncp],
                            in_=ps2[:], func=Copy,
                            scale=dinvp_sb[:, b:b + 1],
                        )
                        if jj == sg - 1:
                            b0 = b - sg + 1
                            nc.sync.dma_start(
                                out=h2_ag_in[b0 * P:(b + 1) * P, :]
                                .rearrange("(g p) c -> p g c", p=P),
                                in_=h2_acc[:].rearrange(
                                    "p (g c) -> p g c", c=ncp),
                            )
                            if b0 * P < lo <= (b + 1) * P:
                                ag_pair(h2_ag_in, h2_lo, h2_hi, "lo")
                    j += gsz
                ag_pair(h2_ag_in, h2_lo, h2_hi, "hi")

            # -------- Phase 3: Agg2 * dinv + b2, log_softmax --------------
            out_acc = None
            jp = 0
            while jp < blocks_per_core:
                gsz = min(gb, blocks_per_core - jp)
                msg2 = gather_group(jp, gsz, h2_lo, h2_hi, ncp, "msg2")
                for kk in range(gsz):
                    b = jp + kk
                    sel = build_selector(b, bf16, "sel")
                    ps_o = psum.tile([P, ncp], f32, tag="psB")
                    for t in range(T):
                        mo = tile_off(kk, gsz, t) * ncp
                        nc.tensor.matmul(
                            out=ps_o[:],
                            lhsT=sel[:, t * P:(t + 1) * P],
                            rhs=msg2[:, mo:mo + ncp],
                            start=(t == 0),
                            stop=(t == T - 1),
                        )
                    u = sb.tile([P, N_CLASSES], f32, tag="u")
                    nc.scalar.activation(
                        out=u[:], in_=ps_o[:, 0:N_CLASSES], func=Copy,
                        scale=dinvp_sb[:, b:b + 1],
                    )
                    logits = sb.tile([P, N_CLASSES], f32, tag="logits")
                    nc.vector.tensor_tensor(
                        out=logits[:], in0=u[:], in1=b2_sb[:],
                        op=mybir.AluOpType.add,
                    )
                    negm = sb.tile([P, 1], f32, tag="negm")
                    nc.vector.reduce_max(
                        out=negm[:], in_=logits[:], axis=mybir.AxisListType.X
                    )
                    nc.vector.tensor_scalar_mul(
                        out=negm[:], in0=negm[:], scalar1=-1.0
                    )
                    expv = sb.tile([P, N_CLASSES], f32, tag="expv")
                    nc.scalar.activation(
                        out=expv[:], in_=logits[:], func=Exp,
                        bias=negm[:, 0:1],
                    )
                    ssum = sb.tile([P, 1], f32, tag="ssum")
                    nc.vector.reduce_sum(
                        out=ssum[:], in_=expv[:], axis=mybir.AxisListType.X
                    )
                    lns = sb.tile([P, 1], f32, tag="lns")
                    nc.scalar.activation(out=lns[:], in_=ssum[:], func=Ln)
                    jj = b % sg
                    if jj == 0:
                        out_acc = sb.tile([P, sg * N_CLASSES], f32,
                                          tag="out_acc", bufs=2)
                    nc.vector.tensor_scalar(
                        out=out_acc[:, jj * N_CLASSES:(jj + 1) * N_CLASSES],
                        in0=logits[:],
                        scalar1=negm[:, 0:1], scalar2=lns[:, 0:1],
                        op0=mybir.AluOpType.add, op1=mybir.AluOpType.subtract,
                    )
                    if jj == sg - 1:
                        b0 = b - sg + 1
                        nc.sync.dma_start(
                            out=out_d[b0 * P:(b + 1) * P, :].rearrange(
                                "(g p) c -> p g c", p=P),
                            in_=out_acc[:].rearrange("p (g c) -> p g c",
                                                     c=N_CLASSES),
                        )
                jp += gsz

    nc.compile()
    return nc


# --------------------------------------------------------------------------
# Host orchestration
# --------------------------------------------------------------------------

def _run(x, edge_index, W1, b1, W2, b2, blocks_per_core):
    from concourse.bass_utils import run_bass_kernel_spmd

    global LAST_RESULT

    x = np.asarray(x, dtype=np.float32)
    W1 = np.asarray(W1, dtype=np.float32)
    b1v = np.asarray(b1, dtype=np.float32).reshape(-1)
    W2 = np.asarray(W2, dtype=np.float32)
    b2v = np.asarray(b2, dtype=np.float32).reshape(-1)

    n_nodes, f_in = x.shape
    hidden = W1.shape[1]
    ncls = W2.shape[1]
    assert hidden == P and ncls == N_CLASSES

    shard = blocks_per_core * P
    n_pad = NCORES * shard

    idxlo, idxhi, segs, dinv_pad, T_lo, T_hi, perm = _preprocess(
        edge_index, n_nodes, blocks_per_core, CH
    )
    T = T_lo + T_hi

    nc = _build_program(f_in, hidden, blocks_per_core, T_lo, T_hi, CH, SG, GB)

    kt = f_in // P
    bf = ml_dtypes.bfloat16

    x_pad = np.zeros((n_pad, f_in), np.float32)
    x_pad[perm] = x
    w1r = np.ascontiguousarray(
        W1.reshape(kt, P, hidden).transpose(1, 0, 2).reshape(P, kt * hidden)
    ).astype(bf)
    w2p = np.zeros((hidden, NCLS_PAD), np.float32)
    w2p[:, :ncls] = W2
    b2t = np.ascontiguousarray(
        np.broadcast_to(b2v[None, :], (P, ncls))
    ).astype(np.float32)
    iotaw = np.ascontiguousarray(
        np.broadcast_to(
            np.tile(np.arange(P, dtype=np.float32), T), (P, T * P)
        )
    ).astype(bf)

    in_maps = []
    for c in range(NCORES):
        xs = x_pad[c * shard:(c + 1) * shard]
        xt4 = np.ascontiguousarray(
            xs.reshape(blocks_per_core, P, kt, P).transpose(3, 0, 2, 1)
            .reshape(P, blocks_per_core * kt * P)
        ).astype(bf)
        dshard = dinv_pad[c * shard:(c + 1) * shard]
        dinvp = np.ascontiguousarray(
            dshard.reshape(blocks_per_core, P).T
        ).astype(np.float32)
        dinvf = np.ascontiguousarray(
            np.broadcast_to(dshard[None, :], (P, shard))
        ).astype(np.float32)
        in_maps.append({
            "xt": xt4,
            "w1": w1r,
            "b1": b1v.reshape(P, 1).copy(),
            "w2": w2p.astype(bf),
            "b2t": b2t,
            "iotaw": iotaw,
            "idxlo": np.ascontiguousarray(idxlo[c]),
            "idxhi": np.ascontiguousarray(idxhi[c]),
            "segs": np.ascontiguousarray(segs[c]).astype(bf),
            "dinvp": dinvp,
            "dinvf": dinvf,
        })

    res = run_bass_kernel_spmd(
        nc, in_maps, core_ids=list(range(NCORES)),
        trace=TRACE, trace_kwargs=dict(TRACE_KWARGS),
    )
    LAST_RESULT = {
        "exec_time_ns": res.exec_time_ns,
        "mean_exec_time_ns": res.mean_exec_time_ns,
        "instructions_and_trace": res.instructions_and_trace,
        "profile_json": res.profile_json,
        "T": T,
        "nc": nc,
        "in_maps": in_maps,
        "perm": perm,
    }
    out = np.concatenate([r["out"] for r in res.results], axis=0)
    return out[perm]


def kernel(x, edge_index, W1, b1, W2, b2):
    n_nodes = np.asarray(x).shape[0]
    blocks_per_core = int(math.ceil(n_nodes / (NCORES * P)))
    return _run(x, edge_index, W1, b1, W2, b2, blocks_per_core)


# revision 50
# speedup vs baseline: 1.2294x; 1.2294x over previous
"""GCN (2-layer, GCNConv + log_softmax) on 8 Trainium2 NeuronCores.

Strategy (1D node partition):
  - Nodes padded to N_PAD = 392*128, sharded contiguously: 49 blocks of 128
    dst-nodes per core.
  - CPU preprocessing: add self-loops, compute dinv=1/sqrt(deg), sort edges by
    dst, pack per (core, block) into edge tiles of 128.  dinv[src] is folded
    into the gathered feature tables (h, h2 stored pre-scaled), dinv[dst] is
    applied after each aggregation, so the selector is a pure one-hot
    (single is_equal on DVE; padding slots get seg=255).
  - Edge->feature gathers use the hardware gather instruction
    (gpsimd.dma_gather / InstDMAGatherAnt, single_packet=False, descriptor
    generation spread over 4 SWDGE queues): amortizes the SWDGE cost over
    GB blocks of edges (the per-tile indirect-DMA variant paid ~1us fixed
    cost per 128 edges and dominated the runtime; multi-column offset APs
    on indirect_dma_start silently misgather on HW).
  - Gather indices are int16, so each layer's feature table is SPLIT in two:
    a "lo" table (per-core rows [0, 4096) -> exactly 32768 rows) and a "hi"
    table (rows [4096, 6272)).  Each table is AllGathered separately, and
    the lo collective is issued as soon as its rows are stored -- overlapping
    the bulk of the collective with the remaining blocks' compute (a single
    Shared tensor may only have one writer, so chunking one table is not
    possible).  Edges are packed into lo tiles then hi tiles per block.
  - On device per core:
      GEMM1: h = (x_shard @ W1) * dinv  (PE bf16, streamed x, ACT-scaled copy)
      AllGather h_lo early, h_hi at phase end -> bf16 h tables [*, 128]
      Agg1 per dst block: lo+hi gathers, one-hot selector, segment-sum via PE
        matmul accumulation into PSUM [hid, dst]; * dinv[dst] (DVE), relu+b1
        (ACT), fused GEMM2 (bf16) -> h2 = (a1 @ W2) * dinv stored bf16
        128-padded (gather rows must be a multiple of 256 bytes).
      AllGather h2_lo early, h2_hi at phase end
      Agg2 per dst block: lo+hi gathers of h2[src], bf16 selector, accumulate
        [dst, 128]; * dinv[dst] + b2 on the first 40 cols, log_softmax; store.
  - Host concatenates the 8 output shards and strips padding.
"""

import math

import numpy as np
import ml_dtypes

P = 128
NCORES = 8
LO_ROWS = 4096          # per-core rows in the "lo" table (8*4096 = 32768,
                        # the int16 gather index limit)

# Full-problem constants (hardcoded per harness contract).
N_NODES = 50000
N_EDGES = 800000
F_IN = 512
HIDDEN = 128
N_CLASSES = 40
NCLS_PAD = 128          # h2 table width: 128 * 2B = 256B rows (bf16)

# Runtime-tunable knobs (test.py may override before calling kernel()).
TRACE = False
TRACE_KWARGS = {}
CH = 1          # AllGather chunks (1 = single collective per layer)
SG = 7          # blocks per store DMA (must divide blocks-per-AG-chunk)
GB = 2          # dst blocks per gather pair (amortizes Q7 gather launch)

LAST_RESULT = {}        # test.py introspection (exec time etc.)


# --------------------------------------------------------------------------
# CPU preprocessing
# --------------------------------------------------------------------------

def _balance_nodes(deg_w, nblk):
    """LPT bin packing: nodes -> blocks of 128, balancing in-edge counts.

    Returns pad_slot[v] = global padded slot (block*128 + row-in-block).
    """
    import heapq

    order_nodes = np.argsort(-deg_w, kind="stable")
    heap = [(0, b) for b in range(nblk)]
    heapq.heapify(heap)
    fill = np.zeros(nblk, np.int64)
    pad_slot = np.empty(len(deg_w), np.int64)
    for v in order_nodes:
        while True:
            load, b = heapq.heappop(heap)
            if fill[b] < P:
                break
        pad_slot[v] = b * P + fill[b]
        fill[b] += 1
        heapq.heappush(heap, (load + int(deg_w[v]), b))
    return pad_slot


def _preprocess(edge_index, n_nodes, blocks_per_core, ch):
    """Sort edges (plus self-loops) by dst; pack per block into lo/hi tiles.

    Returns (idxlo, idxhi, segs, dinv_pad, T_lo, T_hi, perm):
      idxlo [NCORES, 128, BPC*T_lo*8]  int16  gather indices, 16-wrapped
      idxhi [NCORES, 128, BPC*T_hi*8]  int16  (table-local)
      segs  [NCORES, 128, BPC*(T_lo+T_hi)] f32  dst%128 (255 for padding)
      dinv_pad [n_pad] f32
      perm  [n_nodes]  padded global row of each original node
    """
    shard = blocks_per_core * P
    n_pad = NCORES * shard
    qsz = shard // ch
    nblk = NCORES * blocks_per_core
    src = np.asarray(edge_index[0], dtype=np.int64)
    dst = np.asarray(edge_index[1], dtype=np.int64)

    deg = np.bincount(dst, minlength=n_nodes).astype(np.float32) + 1.0
    dinv = (1.0 / np.sqrt(deg)).astype(np.float32)

    # balance per-block edge counts so the uniform tile counts hug the mean
    pad_slot = _balance_nodes(deg.astype(np.int64), nblk)
    blk_of = pad_slot // P
    perm = (blk_of // blocks_per_core) * shard + \
        (blk_of % blocks_per_core) * P + pad_slot % P

    dinv_pad = np.zeros(n_pad, np.float32)
    dinv_pad[perm] = dinv

    loops = np.arange(n_nodes, dtype=np.int64)
    all_src = perm[np.concatenate([src, loops])]
    all_dst = perm[np.concatenate([dst, loops])]

    order = np.argsort(all_dst, kind="stable")
    s_src = all_src[order]
    s_dst = all_dst[order]

    # Two gather tables, each AllGathered separately so the "lo" collective
    # (per-core rows [0, 4096) -> exactly 32768 table rows) can be issued
    # before the phase finishes and overlap with the remaining compute.
    # lo table row of node v = core*4096 + r   (r = local row < 4096)
    # hi table row of node v = core*2176 + (r - 4096)
    c_of = s_src // shard
    r0 = s_src % shard
    is_lo = r0 < LO_ROWS
    s_row = np.where(
        is_lo,
        c_of * LO_ROWS + r0,
        c_of * (shard - LO_ROWS) + (r0 - LO_ROWS),
    ).astype(np.int64)

    blk = s_dst // P

    # per-block lo/hi counts -> uniform tile counts
    cnt_lo = np.bincount(blk[is_lo], minlength=nblk)
    cnt_hi = np.bincount(blk[~is_lo], minlength=nblk)
    T_lo = max(1, int(math.ceil(cnt_lo.max() / P)))
    T_hi = max(1, int(math.ceil(cnt_hi.max() / P)))
    T = T_lo + T_hi

    segs = np.full((NCORES, P, blocks_per_core * T), 255.0, np.float32)
    idxlo16 = np.zeros((NCORES, 16, blocks_per_core * T_lo * 8), np.int16)
    idxhi16 = np.zeros((NCORES, 16, blocks_per_core * T_hi * 8), np.int16)

    for b in range(nblk):
        c, bl = divmod(b, blocks_per_core)
        in_b = blk == b
        for lo_flag, idx_arr, T_part, t_base, w in (
            (True, idxlo16, T_lo, 0, T_lo * 8),
            (False, idxhi16, T_hi, T_lo, T_hi * 8),
        ):
            m = in_b & (is_lo == lo_flag)
            rows = s_row[m]          # already table-local for both tables
            dloc = s_dst[m] % P
            s = np.arange(len(rows))
            # seg for slot s -> tile t_base + s//128, partition s%128
            segs[c, s % P, bl * T + t_base + s // P] = dloc
            # gather idx, 16-wrapped: slot s -> [s%16, s//16]
            idx_arr[c, s % 16, bl * w + s // 16] = rows.astype(np.int16)
    # replicate the 16-partition wrap across all 8 GPSIMD core stripes
    idxlo = np.tile(idxlo16, (1, 8, 1))
    idxhi = np.tile(idxhi16, (1, 8, 1))
    return idxlo, idxhi, segs, dinv_pad, T_lo, T_hi, perm


# --------------------------------------------------------------------------
# Device program
# --------------------------------------------------------------------------

def _build_program(f_in, hidden, blocks_per_core, T_lo, T_hi, ch, sg, gb):
    import concourse.bacc as bacc
    import concourse.mybir as mybir
    import concourse.tile as tile

    dt = mybir.dt
    bf16 = dt.bfloat16
    f32 = dt.float32

    T = T_lo + T_hi
    ncp = NCLS_PAD
    shard = blocks_per_core * P
    n_pad = NCORES * shard
    kt = f_in // P
    qb = blocks_per_core // ch       # blocks per AG chunk
    qsz = qb * P
    wlo, whi = T_lo * 8, T_hi * 8    # idx cols per block
    assert blocks_per_core % ch == 0 and qb % sg == 0

    nc = bacc.Bacc(
        "TRN2",
        target_bir_lowering=False,
        debug=False,
        enable_asserts=False,
        num_devices=NCORES,
        num_swdge_queues=4,
    )

    xt_d = nc.dram_tensor("xt", [P, blocks_per_core * kt * P], bf16,
                          kind="ExternalInput")
    w1_d = nc.dram_tensor("w1", [P, kt * hidden], bf16, kind="ExternalInput")
    b1_d = nc.dram_tensor("b1", [P, 1], f32, kind="ExternalInput")
    w2_d = nc.dram_tensor("w2", [hidden, ncp], bf16, kind="ExternalInput")
    b2_d = nc.dram_tensor("b2t", [P, N_CLASSES], f32, kind="ExternalInput")
    iota_d = nc.dram_tensor("iotaw", [P, T * P], bf16, kind="ExternalInput")
    ilo_d = nc.dram_tensor("idxlo", [P, blocks_per_core * wlo], dt.int16,
                           kind="ExternalInput")
    ihi_d = nc.dram_tensor("idxhi", [P, blocks_per_core * whi], dt.int16,
                           kind="ExternalInput")
    segs_d = nc.dram_tensor("segs", [P, blocks_per_core * T], bf16,
                            kind="ExternalInput")
    dinvp_d = nc.dram_tensor("dinvp", [P, blocks_per_core], f32,
                             kind="ExternalInput")
    dinvf_d = nc.dram_tensor("dinvf", [P, shard], f32, kind="ExternalInput")
    out_d = nc.dram_tensor("out", [shard, N_CLASSES], f32,
                           kind="ExternalOutput")

    RG = [list(range(NCORES))]
    Relu = mybir.ActivationFunctionType.Relu
    Copy = mybir.ActivationFunctionType.Copy
    Exp = mybir.ActivationFunctionType.Exp
    Ln = mybir.ActivationFunctionType.Ln

    with tile.TileContext(nc) as tc:
        with (
            tc.tile_pool(name="const", bufs=1) as const,
            tc.tile_pool(name="dram", bufs=1, space="DRAM") as dram,
            tc.tile_pool(name="sb", bufs=3) as sb,
            tc.tile_pool(name="psum", bufs=2, space="PSUM") as psum,
        ):
            lo, hi = LO_ROWS, shard - LO_ROWS
            h_ag_in = dram.tile([shard, hidden], bf16)
            h_lo = dram.tile([NCORES * lo, hidden], bf16, addr_space="Shared")
            h_hi = dram.tile([NCORES * hi, hidden], bf16, addr_space="Shared")
            h2_ag_in = dram.tile([shard, ncp], bf16)
            h2_lo = dram.tile([NCORES * lo, ncp], bf16, addr_space="Shared")
            h2_hi = dram.tile([NCORES * hi, ncp], bf16, addr_space="Shared")

            w1_sb = const.tile([P, kt * hidden], bf16)
            nc.sync.dma_start(out=w1_sb[:], in_=w1_d[:])
            b1_sb = const.tile([P, 1], f32)
            nc.sync.dma_start(out=b1_sb[:], in_=b1_d[:])
            w2_sb = const.tile([hidden, ncp], bf16)
            nc.sync.dma_start(out=w2_sb[:], in_=w2_d[:])
            b2_sb = const.tile([P, N_CLASSES], f32)
            nc.sync.dma_start(out=b2_sb[:], in_=b2_d[:])
            iota_sb = const.tile([P, T * P], bf16)
            nc.sync.dma_start(out=iota_sb[:], in_=iota_d[:])
            ilo_sb = const.tile([P, blocks_per_core * wlo], dt.int16)
            nc.sync.dma_start(out=ilo_sb[:], in_=ilo_d[:])
            ihi_sb = const.tile([P, blocks_per_core * whi], dt.int16)
            nc.sync.dma_start(out=ihi_sb[:], in_=ihi_d[:])
            segs_sb = const.tile([P, blocks_per_core * T], bf16)
            nc.sync.dma_start(out=segs_sb[:], in_=segs_d[:])
            dinvp_sb = const.tile([P, blocks_per_core], f32)
            nc.sync.dma_start(out=dinvp_sb[:], in_=dinvp_d[:])
            dinvf_sb = const.tile([P, shard], f32)
            nc.sync.dma_start(out=dinvf_sb[:], in_=dinvf_d[:])

            iota3 = iota_sb[:].rearrange("p (t d) -> p t d", d=P)

            def build_selector(b, seldt, tag):
                g0 = b * T
                sel = sb.tile([P, T * P], seldt, tag=tag)
                nc.vector.tensor_tensor(
                    out=sel[:].rearrange("p (t d) -> p t d", d=P),
                    in0=iota3,
                    in1=segs_sb[:, g0:g0 + T].to_broadcast([P, T, P]),
                    op=mybir.AluOpType.is_equal,
                )
                return sel

            gq = [0]  # rotating SWDGE queue assignment

            def gather_group(b0, gsz, t_lo, t_hi, F, tag):
                """One lo + one hi gather covering blocks [b0, b0+gsz)."""
                msg = sb.tile([P, gb * T * F], bf16, tag=tag, bufs=2)
                nc.gpsimd.dma_gather(
                    out_ap=msg[:, :gsz * T_lo * F].rearrange(
                        "p (t f) -> p t f", f=F),
                    in_ap=t_lo[:],
                    idxs_ap=ilo_sb[:, b0 * wlo:(b0 + gsz) * wlo],
                    num_idxs=gsz * T_lo * P,
                    num_idxs_reg=gsz * T_lo * P,
                    elem_size=F,
                    single_packet=False,
                    queue_num=gq[0],
                )
                gq[0] = (gq[0] + 1) % 4
                nc.gpsimd.dma_gather(
                    out_ap=msg[:, gsz * T_lo * F:gsz * T * F].rearrange(
                        "p (t f) -> p t f", f=F),
                    in_ap=t_hi[:],
                    idxs_ap=ihi_sb[:, b0 * whi:(b0 + gsz) * whi],
                    num_idxs=gsz * T_hi * P,
                    num_idxs_reg=gsz * T_hi * P,
                    elem_size=F,
                    single_packet=False,
                    queue_num=gq[0],
                )
                gq[0] = (gq[0] + 1) % 4
                return msg

            def ag_pair(src_dram, out_lo, out_hi, which):
                """Issue the lo or hi AllGather of a phase's table."""
                if which == "lo":
                    ins, outs = src_dram[0:lo, :], out_lo[:]
                else:
                    ins, outs = src_dram[lo:shard, :], out_hi[:]
                nc.gpsimd.collective_compute(
                    "AllGather",
                    mybir.AluOpType.bypass,
                    replica_groups=RG,
                    ins=[ins],
                    outs=[outs],
                )

            def tile_off(k, gsz, t):
                """msg tile index of block k's selector tile t in a group."""
                if t < T_lo:
                    return k * T_lo + t
                return gsz * T_lo + k * T_hi + (t - T_lo)

            # ---------------- Phase 1: GEMM1 (h = (x @ W1) * dinv) --------
            for q in range(ch):
                h_acc = None
                xt_t = None
                for j in range(qb):
                    i = q * qb + j
                    jj = j % sg
                    if jj == 0:
                        xt_t = sb.tile([P, sg * kt * P], bf16, tag="xt_t",
                                       bufs=2)
                        nc.sync.dma_start(
                            out=xt_t[:],
                            in_=xt_d[:, i * kt * P:(i + sg) * kt * P],
                        )
                    ps = psum.tile([P, P], f32, tag="psA")
                    for k in range(kt):
                        c0 = (jj * kt + k) * P
                        nc.tensor.matmul(
                            out=ps[:],
                            lhsT=xt_t[:, c0:c0 + P],
                            rhs=w1_sb[:, k * hidden:(k + 1) * hidden],
                            start=(k == 0),
                            stop=(k == kt - 1),
                        )
                    if jj == 0:
                        h_acc = sb.tile([P, sg * hidden], bf16, tag="h_acc",
                                        bufs=2)
                    nc.scalar.activation(
                        out=h_acc[:, jj * hidden:(jj + 1) * hidden],
                        in_=ps[:], func=Copy, scale=dinvp_sb[:, i:i + 1],
                    )
                    if jj == sg - 1:
                        b0 = i - sg + 1
                        nc.sync.dma_start(
                            out=h_ag_in[b0 * P:(i + 1) * P, :].rearrange(
                                "(g p) h -> p g h", p=P),
                            in_=h_acc[:].rearrange("p (g h) -> p g h",
                                                   h=hidden),
                        )
                        if b0 * P < lo <= (i + 1) * P:
                            # lo-table rows complete: start its AllGather
                            # while the remaining blocks compute.
                            ag_pair(h_ag_in, h_lo, h_hi, "lo")
                ag_pair(h_ag_in, h_lo, h_hi, "hi")

            # ------- Phase 2: Agg1 * dinv, relu + b1, GEMM2, * dinv -------
            for q in range(ch):
                h2_acc = None
                j = 0
                while j < qb:
                    gsz = min(gb, qb - j)
                    bg = q * qb + j
                    msg = gather_group(bg, gsz, h_lo, h_hi, hidden, "msg")
                    for kk in range(gsz):
                        b = bg + kk
                        sel = build_selector(b, bf16, "sel")
                        ps1 = psum.tile([P, P], f32, tag="psA")
                        for t in range(T):
                            mo = tile_off(kk, gsz, t) * hidden
                            nc.tensor.matmul(
                                out=ps1[:],
                                lhsT=msg[:, mo:mo + hidden],
                                rhs=sel[:, t * P:(t + 1) * P],
                                start=(t == 0),
                                stop=(t == T - 1),
                            )
                        # ps1 = agg1^T : [hid, dst]; * dinv[dst], relu(.+b1)
                        t1 = sb.tile([P, P], f32, tag="t1")
                        nc.vector.tensor_tensor(
                            out=t1[:], in0=ps1[:],
                            in1=dinvf_sb[:, b * P:(b + 1) * P],
                            op=mybir.AluOpType.mult,
                        )
                        a1 = sb.tile([P, P], bf16, tag="a1")
                        nc.scalar.activation(
                            out=a1[:], in_=t1[:], func=Relu,
                            bias=b1_sb[:, 0:1],
                        )
                        ps2 = psum.tile([P, ncp], f32, tag="psB")
                        nc.tensor.matmul(
                            out=ps2[:], lhsT=a1[:], rhs=w2_sb[:],
                            start=True, stop=True,
                        )
                        jj = (j + kk) % sg
                        if jj == 0:
                            h2_acc = sb.tile([P, sg * ncp], bf16,
                                             tag="h2_acc", bufs=2)
                        nc.scalar.activation(
                            out=h2_acc[:, jj * ncp:(jj + 1) * ncp],
                            in_=ps2[:], func=Copy,
                            scale=dinvp_sb[:, b:b + 1],
                        )
                        if jj == sg - 1:
                            b0 = b - sg + 1
                            nc.sync.dma_start(
                                out=h2_ag_in[b0 * P:(b + 1) * P, :]
                                .rearrange("(g p) c -> p g c", p=P),
                                in_=h2_acc[:].rearrange(
                                    "p (g c) -> p g c", c=ncp),
                            )
                            if b0 * P < lo <= (b + 1) * P:
                                ag_pair(h2_ag_in, h2_lo, h2_hi, "lo")
                    j += gsz
                ag_pair(h2_ag_in, h2_lo, h2_hi, "hi")

            # -------- Phase 3: Agg2 * dinv + b2, log_softmax --------------
            out_acc = None
            jp = 0
            while jp < blocks_per_core:
                gsz = min(gb, blocks_per_core - jp)
                msg2 = gather_group(jp, gsz, h2_lo, h2_hi, ncp, "msg2")
                for kk in range(gsz):
                    b = jp + kk
                    sel = build_selector(b, bf16, "sel")
                    ps_o = psum.tile([P, ncp], f32, tag="psB")
                    for t in range(T):
                        mo = tile_off(kk, gsz, t) * ncp
                        nc.tensor.matmul(
                            out=ps_o[:],
                            lhsT=sel[:, t * P:(t + 1) * P],
                            rhs=msg2[:, mo:mo + ncp],
                            start=(t == 0),
                            stop=(t == T - 1),
                        )
                    u = sb.tile([P, N_CLASSES], f32, tag="u")
                    nc.scalar.activation(
                        out=u[:], in_=ps_o[:, 0:N_CLASSES], func=Copy,
                        scale=dinvp_sb[:, b:b + 1],
                    )
                    logits = sb.tile([P, N_CLASSES], f32, tag="logits")
                    nc.vector.tensor_tensor(
                        out=logits[:], in0=u[:], in1=b2_sb[:],
                        op=mybir.AluOpType.add,
                    )
                    negm = sb.tile([P, 1], f32, tag="negm")
                    nc.vector.reduce_max(
                        out=negm[:], in_=logits[:], axis=mybir.AxisListType.X
                    )
                    nc.vector.tensor_scalar_mul(
                        out=negm[:], in0=negm[:], scalar1=-1.0
                    )
                    expv = sb.tile([P, N_CLASSES], f32, tag="expv")
                    nc.scalar.activation(
                        out=expv[:], in_=logits[:], func=Exp,
                        bias=negm[:, 0:1],
                    )
                    ssum = sb.tile([P, 1], f32, tag="ssum")
                    nc.vector.reduce_sum(
                        out=ssum[:], in_=expv[:], axis=mybir.AxisListType.X
                    )
                    lns = sb.tile([P, 1], f32, tag="lns")
                    nc.scalar.activation(out=lns[:], in_=ssum[:], func=Ln)
                    jj = b % sg
                    if jj == 0:
                        out_acc = sb.tile([P, sg * N_CLASSES], f32,
                                          tag="out_acc", bufs=2)
                    nc.vector.tensor_scalar(
                        out=out_acc[:, jj * N_CLASSES:(jj + 1) * N_CLASSES],
                        in0=logits[:],
                        scalar1=negm[:, 0:1], scalar2=lns[:, 0:1],
                        op0=mybir.AluOpType.add, op1=mybir.AluOpType.subtract,
                    )
                    if jj == sg - 1:
                        b0 = b - sg + 1
                        nc.sync.dma_start(
                            out=out_d[b0 * P:(b + 1) * P, :].rearrange(
                                "(g p) c -> p g c", p=P),
                            in_=out_acc[:].rearrange("p (g c) -> p g c",
                                                     c=N_CLASSES),
                        )
                jp += gsz

    nc.compile()
    return nc


# --------------------------------------------------------------------------
# Host orchestration
# --------------------------------------------------------------------------

def _run(x, edge_index, W1, b1, W2, b2, blocks_per_core):
    from concourse.bass_utils import run_bass_kernel_spmd

    global LAST_RESULT

    x = np.asarray(x, dtype=np.float32)
    W1 = np.asarray(W1, dtype=np.float32)
    b1v = np.asarray(b1, dtype=np.float32).reshape(-1)
    W2 = np.asarray(W2, dtype=np.float32)
    b2v = np.asarray(b2, dtype=np.float32).reshape(-1)

    n_nodes, f_in = x.shape
    hidden = W1.shape[1]
    ncls = W2.shape[1]
    assert hidden == P and ncls == N_CLASSES

    shard = blocks_per_core * P
    n_pad = NCORES * shard

    idxlo, idxhi, segs, dinv_pad, T_lo, T_hi, perm = _preprocess(
        edge_index, n_nodes, blocks_per_core, CH
    )
    T = T_lo + T_hi

    nc = _build_program(f_in, hidden, blocks_per_core, T_lo, T_hi, CH, SG, GB)

    kt = f_in // P
    bf = ml_dtypes.bfloat16

    x_pad = np.zeros((n_pad, f_in), np.float32)
    x_pad[perm] = x
    w1r = np.ascontiguousarray(
        W1.reshape(kt, P, hidden).transpose(1, 0, 2).reshape(P, kt * hidden)
    ).astype(bf)
    w2p = np.zeros((hidden, NCLS_PAD), np.float32)
    w2p[:, :ncls] = W2
    b2t = np.ascontiguousarray(
        np.broadcast_to(b2v[None, :], (P, ncls))
    ).astype(np.float32)
    iotaw = np.ascontiguousarray(
        np.broadcast_to(
            np.tile(np.arange(P, dtype=np.float32), T), (P, T * P)
        )
    ).astype(bf)

    in_maps = []
    for c in range(NCORES):
        xs = x_pad[c * shard:(c + 1) * shard]
        xt4 = np.ascontiguousarray(
            xs.reshape(blocks_per_core, P, kt, P).transpose(3, 0, 2, 1)
            .reshape(P, blocks_per_core * kt * P)
        ).astype(bf)
        dshard = dinv_pad[c * shard:(c + 1) * shard]
        dinvp = np.ascontiguousarray(
            dshard.reshape(blocks_per_core, P).T
        ).astype(np.float32)
        dinvf = np.ascontiguousarray(
            np.broadcast_to(dshard[None, :], (P, shard))
        ).astype(np.float32)
        in_maps.append({
            "xt": xt4,
            "w1": w1r,
            "b1": b1v.reshape(P, 1).copy(),
            "w2": w2p.astype(bf),
            "b2t": b2t,
            "iotaw": iotaw,
            "idxlo": np.ascontiguousarray(idxlo[c]),
            "idxhi": np.ascontiguousarray(idxhi[c]),
            "segs": np.ascontiguousarray(segs[c]).astype(bf),
            "dinvp": dinvp,
            "dinvf": dinvf,
        })

    res = run_bass_kernel_spmd(
        nc, in_maps, core_ids=list(range(NCORES)),
        trace=TRACE, trace_kwargs=dict(TRACE_KWARGS),
    )
    LAST_RESULT = {
        "exec_time_ns": res.exec_time_ns,
        "mean_exec_time_ns": res.mean_exec_time_ns,
        "instructions_and_trace": res.instructions_and_trace,
        "profile_json": res.profile_json,
        "T": T,
        "nc": nc,
        "in_maps": in_maps,
        "perm": perm,
    }
    out = np.concatenate([r["out"] for r in res.results], axis=0)
    return out[perm]


def kernel(x, edge_index, W1, b1, W2, b2):
    n_nodes = np.asarray(x).shape[0]
    blocks_per_core = int(math.ceil(n_nodes / (NCORES * P)))
    return _run(x, edge_index, W1, b1, W2, b2, blocks_per_core)


# revision 51
# speedup vs baseline: 1.4514x; 1.1806x over previous
"""GCN (2-layer, GCNConv + log_softmax) on 8 Trainium2 NeuronCores.

Strategy (1D node partition):
  - Nodes padded to N_PAD = 392*128, sharded contiguously: 49 blocks of 128
    dst-nodes per core.
  - CPU preprocessing: add self-loops, compute dinv=1/sqrt(deg), sort edges by
    dst, pack per (core, block) into edge tiles of 128.  dinv[src] is folded
    into the gathered feature tables (h, h2 stored pre-scaled), dinv[dst] is
    applied after each aggregation, so the selector is a pure one-hot
    (single is_equal on DVE; padding slots get seg=255).
  - Edge->feature gathers use the hardware gather instruction
    (gpsimd.dma_gather / InstDMAGatherAnt, single_packet=False, descriptor
    generation spread over 4 SWDGE queues): amortizes the SWDGE cost over
    GB blocks of edges (the per-tile indirect-DMA variant paid ~1us fixed
    cost per 128 edges and dominated the runtime; multi-column offset APs
    on indirect_dma_start silently misgather on HW).
  - Gather indices are int16, so each layer's feature table is SPLIT in two:
    a "lo" table (per-core rows [0, 4096) -> exactly 32768 rows) and a "hi"
    table (rows [4096, 6272)).  Each table is AllGathered separately, and
    the lo collective is issued as soon as its rows are stored -- overlapping
    the bulk of the collective with the remaining blocks' compute (a single
    Shared tensor may only have one writer, so chunking one table is not
    possible).  Edges are packed into lo tiles then hi tiles per block.
  - On device per core:
      GEMM1: h = (x_shard @ W1) * dinv  (PE bf16, streamed x, ACT-scaled copy)
      AllGather h_lo early, h_hi at phase end -> bf16 h tables [*, 128]
      Agg1 per dst block: lo+hi gathers, one-hot selector, segment-sum via PE
        matmul accumulation into PSUM [hid, dst]; * dinv[dst] (DVE), relu+b1
        (ACT), fused GEMM2 (bf16) -> h2 = (a1 @ W2) * dinv stored bf16
        128-padded (gather rows must be a multiple of 256 bytes).
      AllGather h2_lo early, h2_hi at phase end
      Agg2 per dst block: lo+hi gathers of h2[src], bf16 selector, accumulate
        [dst, 128]; * dinv[dst] + b2 on the first 40 cols, log_softmax; store.
  - Host concatenates the 8 output shards and strips padding.
"""

import math

import numpy as np
import ml_dtypes

P = 128
NCORES = 8
LO_ROWS = 4096          # per-core rows in the "lo" table (8*4096 = 32768,
                        # the int16 gather index limit)

# Full-problem constants (hardcoded per harness contract).
N_NODES = 50000
N_EDGES = 800000
F_IN = 512
HIDDEN = 128
N_CLASSES = 40
NCLS_PAD = 128          # h2 table width: 128 * 2B = 256B rows (bf16)

# Runtime-tunable knobs (test.py may override before calling kernel()).
TRACE = False
TRACE_KWARGS = {}
CH = 1          # AllGather chunks (1 = single collective per layer)
SG = 7          # blocks per store DMA (must divide blocks-per-AG-chunk)
GB = 2          # dst blocks per gather pair (amortizes Q7 gather launch)

LAST_RESULT = {}        # test.py introspection (exec time etc.)


# --------------------------------------------------------------------------
# CPU preprocessing
# --------------------------------------------------------------------------

def _balance_nodes(deg_w, nblk):
    """LPT bin packing: nodes -> blocks of 128, balancing in-edge counts.

    Returns pad_slot[v] = global padded slot (block*128 + row-in-block).
    """
    import heapq

    order_nodes = np.argsort(-deg_w, kind="stable")
    heap = [(0, b) for b in range(nblk)]
    heapq.heapify(heap)
    fill = np.zeros(nblk, np.int64)
    pad_slot = np.empty(len(deg_w), np.int64)
    for v in order_nodes:
        while True:
            load, b = heapq.heappop(heap)
            if fill[b] < P:
                break
        pad_slot[v] = b * P + fill[b]
        fill[b] += 1
        heapq.heappush(heap, (load + int(deg_w[v]), b))
    return pad_slot


def _preprocess(edge_index, n_nodes, blocks_per_core, ch):
    """Sort edges (plus self-loops) by dst; pack per block into lo/hi tiles.

    Returns (idxlo, idxhi, segs, dinv_pad, T_lo, T_hi, perm):
      idxlo [NCORES, 128, BPC*T_lo*8]  int16  gather indices, 16-wrapped
      idxhi [NCORES, 128, BPC*T_hi*8]  int16  (table-local)
      segs  [NCORES, 128, BPC*(T_lo+T_hi)] f32  dst%128 (255 for padding)
      dinv_pad [n_pad] f32
      perm  [n_nodes]  padded global row of each original node
    """
    shard = blocks_per_core * P
    n_pad = NCORES * shard
    qsz = shard // ch
    nblk = NCORES * blocks_per_core
    src = np.asarray(edge_index[0], dtype=np.int64)
    dst = np.asarray(edge_index[1], dtype=np.int64)

    deg = np.bincount(dst, minlength=n_nodes).astype(np.float32) + 1.0
    dinv = (1.0 / np.sqrt(deg)).astype(np.float32)

    # balance per-block edge counts so the uniform tile counts hug the mean
    pad_slot = _balance_nodes(deg.astype(np.int64), nblk)
    blk_of = pad_slot // P
    perm = (blk_of // blocks_per_core) * shard + \
        (blk_of % blocks_per_core) * P + pad_slot % P

    dinv_pad = np.zeros(n_pad, np.float32)
    dinv_pad[perm] = dinv

    loops = np.arange(n_nodes, dtype=np.int64)
    all_src = perm[np.concatenate([src, loops])]
    all_dst = perm[np.concatenate([dst, loops])]

    order = np.argsort(all_dst, kind="stable")
    s_src = all_src[order]
    s_dst = all_dst[order]

    # Two gather tables, each AllGathered separately so the "lo" collective
    # (per-core rows [0, 4096) -> exactly 32768 table rows) can be issued
    # before the phase finishes and overlap with the remaining compute.
    # lo table row of node v = core*4096 + r   (r = local row < 4096)
    # hi table row of node v = core*2176 + (r - 4096)
    c_of = s_src // shard
    r0 = s_src % shard
    is_lo = r0 < LO_ROWS
    s_row = np.where(
        is_lo,
        c_of * LO_ROWS + r0,
        c_of * (shard - LO_ROWS) + (r0 - LO_ROWS),
    ).astype(np.int64)

    blk = s_dst // P

    # per-block lo/hi counts -> uniform tile counts
    cnt_lo = np.bincount(blk[is_lo], minlength=nblk)
    cnt_hi = np.bincount(blk[~is_lo], minlength=nblk)
    T_lo = max(1, int(math.ceil(cnt_lo.max() / P)))
    T_hi = max(1, int(math.ceil(cnt_hi.max() / P)))
    T = T_lo + T_hi

    segs = np.full((NCORES, P, blocks_per_core * T), 255.0, np.float32)
    idxlo16 = np.zeros((NCORES, 16, blocks_per_core * T_lo * 8), np.int16)
    idxhi16 = np.zeros((NCORES, 16, blocks_per_core * T_hi * 8), np.int16)

    for b in range(nblk):
        c, bl = divmod(b, blocks_per_core)
        in_b = blk == b
        for lo_flag, idx_arr, T_part, t_base, w in (
            (True, idxlo16, T_lo, 0, T_lo * 8),
            (False, idxhi16, T_hi, T_lo, T_hi * 8),
        ):
            m = in_b & (is_lo == lo_flag)
            rows = s_row[m]          # already table-local for both tables
            dloc = s_dst[m] % P
            s = np.arange(len(rows))
            # seg for slot s -> tile t_base + s//128, partition s%128
            segs[c, s % P, bl * T + t_base + s // P] = dloc
            # gather idx, 16-wrapped: slot s -> [s%16, s//16]
            idx_arr[c, s % 16, bl * w + s // 16] = rows.astype(np.int16)
    # replicate the 16-partition wrap across all 8 GPSIMD core stripes
    idxlo = np.tile(idxlo16, (1, 8, 1))
    idxhi = np.tile(idxhi16, (1, 8, 1))
    return idxlo, idxhi, segs, dinv_pad, T_lo, T_hi, perm


# --------------------------------------------------------------------------
# Device program
# --------------------------------------------------------------------------

def _build_program(f_in, hidden, blocks_per_core, T_lo, T_hi, ch, sg, gb):
    import concourse.bacc as bacc
    import concourse.mybir as mybir
    import concourse.tile as tile

    dt = mybir.dt
    bf16 = dt.bfloat16
    f32 = dt.float32

    T = T_lo + T_hi
    ncp = NCLS_PAD
    shard = blocks_per_core * P
    n_pad = NCORES * shard
    kt = f_in // P
    qb = blocks_per_core // ch       # blocks per AG chunk
    qsz = qb * P
    wlo, whi = T_lo * 8, T_hi * 8    # idx cols per block
    assert blocks_per_core % ch == 0 and qb % sg == 0

    nc = bacc.Bacc(
        "TRN2",
        target_bir_lowering=False,
        debug=False,
        enable_asserts=False,
        num_devices=NCORES,
        num_swdge_queues=4,
    )

    xt_d = nc.dram_tensor("xt", [P, blocks_per_core * kt * P], bf16,
                          kind="ExternalInput")
    w1_d = nc.dram_tensor("w1", [P, kt * hidden], bf16, kind="ExternalInput")
    b1_d = nc.dram_tensor("b1", [P, 1], f32, kind="ExternalInput")
    w2_d = nc.dram_tensor("w2", [hidden, ncp], bf16, kind="ExternalInput")
    b2_d = nc.dram_tensor("b2t", [P, N_CLASSES], f32, kind="ExternalInput")
    iota_d = nc.dram_tensor("iotaw", [P, T * P], bf16, kind="ExternalInput")
    ilo_d = nc.dram_tensor("idxlo", [P, blocks_per_core * wlo], dt.int16,
                           kind="ExternalInput")
    ihi_d = nc.dram_tensor("idxhi", [P, blocks_per_core * whi], dt.int16,
                           kind="ExternalInput")
    segs_d = nc.dram_tensor("segs", [P, blocks_per_core * T], bf16,
                            kind="ExternalInput")
    dinvp_d = nc.dram_tensor("dinvp", [P, blocks_per_core], f32,
                             kind="ExternalInput")
    dinvf_d = nc.dram_tensor("dinvf", [P, shard], f32, kind="ExternalInput")
    out_d = nc.dram_tensor("out", [shard, N_CLASSES], f32,
                           kind="ExternalOutput")

    RG = [list(range(NCORES))]
    Relu = mybir.ActivationFunctionType.Relu
    Copy = mybir.ActivationFunctionType.Copy
    Exp = mybir.ActivationFunctionType.Exp
    Ln = mybir.ActivationFunctionType.Ln

    with tile.TileContext(nc) as tc:
        with (
            tc.tile_pool(name="const", bufs=1) as const,
            tc.tile_pool(name="dram", bufs=1, space="DRAM") as dram,
            tc.tile_pool(name="sb", bufs=3) as sb,
            tc.tile_pool(name="psum", bufs=2, space="PSUM") as psum,
        ):
            lo, hi = LO_ROWS, shard - LO_ROWS
            h_ag_in = dram.tile([shard, hidden], bf16)
            h_lo = dram.tile([NCORES * lo, hidden], bf16, addr_space="Shared")
            h_hi = dram.tile([NCORES * hi, hidden], bf16, addr_space="Shared")
            h2_ag_in = dram.tile([shard, ncp], bf16)
            h2_lo = dram.tile([NCORES * lo, ncp], bf16, addr_space="Shared")
            h2_hi = dram.tile([NCORES * hi, ncp], bf16, addr_space="Shared")

            w1_sb = const.tile([P, kt * hidden], bf16)
            nc.sync.dma_start(out=w1_sb[:], in_=w1_d[:])
            b1_sb = const.tile([P, 1], f32)
            nc.sync.dma_start(out=b1_sb[:], in_=b1_d[:])
            w2_sb = const.tile([hidden, ncp], bf16)
            nc.sync.dma_start(out=w2_sb[:], in_=w2_d[:])
            b2_sb = const.tile([P, N_CLASSES], f32)
            nc.sync.dma_start(out=b2_sb[:], in_=b2_d[:])
            iota_sb = const.tile([P, T * P], bf16)
            nc.sync.dma_start(out=iota_sb[:], in_=iota_d[:])
            ilo_sb = const.tile([P, blocks_per_core * wlo], dt.int16)
            nc.sync.dma_start(out=ilo_sb[:], in_=ilo_d[:])
            ihi_sb = const.tile([P, blocks_per_core * whi], dt.int16)
            nc.sync.dma_start(out=ihi_sb[:], in_=ihi_d[:])
            segs_sb = const.tile([P, blocks_per_core * T], bf16)
            nc.sync.dma_start(out=segs_sb[:], in_=segs_d[:])
            dinvp_sb = const.tile([P, blocks_per_core], f32)
            nc.sync.dma_start(out=dinvp_sb[:], in_=dinvp_d[:])
            dinvf_sb = const.tile([P, shard], f32)
            nc.sync.dma_start(out=dinvf_sb[:], in_=dinvf_d[:])

            iota3 = iota_sb[:].rearrange("p (t d) -> p t d", d=P)

            def build_selector(b, seldt, tag):
                g0 = b * T
                sel = sb.tile([P, T * P], seldt, tag=tag)
                nc.vector.tensor_tensor(
                    out=sel[:].rearrange("p (t d) -> p t d", d=P),
                    in0=iota3,
                    in1=segs_sb[:, g0:g0 + T].to_broadcast([P, T, P]),
                    op=mybir.AluOpType.is_equal,
                )
                return sel

            gq = [0]  # rotating SWDGE queue assignment

            def gather_group(b0, gsz, t_lo, t_hi, F, tag):
                """One lo + one hi gather covering blocks [b0, b0+gsz)."""
                msg = sb.tile([P, gb * T * F], bf16, tag=tag, bufs=3)
                nc.gpsimd.dma_gather(
                    out_ap=msg[:, :gsz * T_lo * F].rearrange(
                        "p (t f) -> p t f", f=F),
                    in_ap=t_lo[:],
                    idxs_ap=ilo_sb[:, b0 * wlo:(b0 + gsz) * wlo],
                    num_idxs=gsz * T_lo * P,
                    num_idxs_reg=gsz * T_lo * P,
                    elem_size=F,
                    single_packet=False,
                    queue_num=gq[0],
                )
                gq[0] = (gq[0] + 1) % 4
                nc.gpsimd.dma_gather(
                    out_ap=msg[:, gsz * T_lo * F:gsz * T * F].rearrange(
                        "p (t f) -> p t f", f=F),
                    in_ap=t_hi[:],
                    idxs_ap=ihi_sb[:, b0 * whi:(b0 + gsz) * whi],
                    num_idxs=gsz * T_hi * P,
                    num_idxs_reg=gsz * T_hi * P,
                    elem_size=F,
                    single_packet=False,
                    queue_num=gq[0],
                )
                gq[0] = (gq[0] + 1) % 4
                return msg

            def ag_pair(src_dram, out_lo, out_hi, which):
                """Issue the lo or hi AllGather of a phase's table."""
                if which == "lo":
                    ins, outs = src_dram[0:lo, :], out_lo[:]
                else:
                    ins, outs = src_dram[lo:shard, :], out_hi[:]
                nc.gpsimd.collective_compute(
                    "AllGather",
                    mybir.AluOpType.bypass,
                    replica_groups=RG,
                    ins=[ins],
                    outs=[outs],
                )

            def tile_off(k, gsz, t):
                """msg tile index of block k's selector tile t in a group."""
                if t < T_lo:
                    return k * T_lo + t
                return gsz * T_lo + k * T_hi + (t - T_lo)

            # ---------------- Phase 1: GEMM1 (h = (x @ W1) * dinv) --------
            for q in range(ch):
                h_acc = None
                xt_t = None
                for j in range(qb):
                    i = q * qb + j
                    jj = j % sg
                    if jj == 0:
                        xt_t = sb.tile([P, sg * kt * P], bf16, tag="xt_t",
                                       bufs=2)
                        nc.sync.dma_start(
                            out=xt_t[:],
                            in_=xt_d[:, i * kt * P:(i + sg) * kt * P],
                        )
                    ps = psum.tile([P, P], f32, tag="psA")
                    for k in range(kt):
                        c0 = (jj * kt + k) * P
                        nc.tensor.matmul(
                            out=ps[:],
                            lhsT=xt_t[:, c0:c0 + P],
                            rhs=w1_sb[:, k * hidden:(k + 1) * hidden],
                            start=(k == 0),
                            stop=(k == kt - 1),
                        )
                    if jj == 0:
                        h_acc = sb.tile([P, sg * hidden], bf16, tag="h_acc",
                                        bufs=2)
                    nc.scalar.activation(
                        out=h_acc[:, jj * hidden:(jj + 1) * hidden],
                        in_=ps[:], func=Copy, scale=dinvp_sb[:, i:i + 1],
                    )
                    if jj == sg - 1:
                        b0 = i - sg + 1
                        nc.sync.dma_start(
                            out=h_ag_in[b0 * P:(i + 1) * P, :].rearrange(
                                "(g p) h -> p g h", p=P),
                            in_=h_acc[:].rearrange("p (g h) -> p g h",
                                                   h=hidden),
                        )
                        if b0 * P < lo <= (i + 1) * P:
                            # lo-table rows complete: start its AllGather
                            # while the remaining blocks compute.
                            ag_pair(h_ag_in, h_lo, h_hi, "lo")
                ag_pair(h_ag_in, h_lo, h_hi, "hi")

            # ------- Phase 2: Agg1 * dinv, relu + b1, GEMM2, * dinv -------
            for q in range(ch):
                h2_acc = None
                j = 0
                while j < qb:
                    gsz = min(gb, qb - j)
                    bg = q * qb + j
                    msg = gather_group(bg, gsz, h_lo, h_hi, hidden, "msg")
                    for kk in range(gsz):
                        b = bg + kk
                        sel = build_selector(b, bf16, "sel")
                        ps1 = psum.tile([P, P], f32, tag="psA")
                        for t in range(T):
                            mo = tile_off(kk, gsz, t) * hidden
                            nc.tensor.matmul(
                                out=ps1[:],
                                lhsT=msg[:, mo:mo + hidden],
                                rhs=sel[:, t * P:(t + 1) * P],
                                start=(t == 0),
                                stop=(t == T - 1),
                            )
                        # ps1 = agg1^T : [hid, dst]; * dinv[dst], relu(.+b1)
                        t1 = sb.tile([P, P], f32, tag="t1")
                        nc.vector.tensor_tensor(
                            out=t1[:], in0=ps1[:],
                            in1=dinvf_sb[:, b * P:(b + 1) * P],
                            op=mybir.AluOpType.mult,
                        )
                        a1 = sb.tile([P, P], bf16, tag="a1")
                        nc.scalar.activation(
                            out=a1[:], in_=t1[:], func=Relu,
                            bias=b1_sb[:, 0:1],
                        )
                        ps2 = psum.tile([P, ncp], f32, tag="psB")
                        nc.tensor.matmul(
                            out=ps2[:], lhsT=a1[:], rhs=w2_sb[:],
                            start=True, stop=True,
                        )
                        jj = (j + kk) % sg
                        if jj == 0:
                            h2_acc = sb.tile([P, sg * ncp], bf16,
                                             tag="h2_acc", bufs=2)
                        nc.scalar.activation(
                            out=h2_acc[:, jj * ncp:(jj + 1) * ncp],
                            in_=ps2[:], func=Copy,
                            scale=dinvp_sb[:, b:b + 1],
                        )
                        if jj == sg - 1:
                            b0 = b - sg + 1
                            nc.sync.dma_start(
                                out=h2_ag_in[b0 * P:(b + 1) * P, :]
                                .rearrange("(g p) c -> p g c", p=P),
                                in_=h2_acc[:].rearrange(
                                    "p (g c) -> p g c", c=ncp),
                            )
                            if b0 * P < lo <= (b + 1) * P:
                                ag_pair(h2_ag_in, h2_lo, h2_hi, "lo")
                    j += gsz
                ag_pair(h2_ag_in, h2_lo, h2_hi, "hi")

            # -------- Phase 3: Agg2 * dinv + b2, log_softmax --------------
            out_acc = None
            jp = 0
            while jp < blocks_per_core:
                gsz = min(gb, blocks_per_core - jp)
                msg2 = gather_group(jp, gsz, h2_lo, h2_hi, ncp, "msg2")
                for kk in range(gsz):
                    b = jp + kk
                    sel = build_selector(b, bf16, "sel")
                    ps_o = psum.tile([P, ncp], f32, tag="psB")
                    for t in range(T):
                        mo = tile_off(kk, gsz, t) * ncp
                        nc.tensor.matmul(
                            out=ps_o[:],
                            lhsT=sel[:, t * P:(t + 1) * P],
                            rhs=msg2[:, mo:mo + ncp],
                            start=(t == 0),
                            stop=(t == T - 1),
                        )
                    u = sb.tile([P, N_CLASSES], f32, tag="u")
                    nc.scalar.activation(
                        out=u[:], in_=ps_o[:, 0:N_CLASSES], func=Copy,
                        scale=dinvp_sb[:, b:b + 1],
                    )
                    logits = sb.tile([P, N_CLASSES], f32, tag="logits")
                    nc.vector.tensor_tensor(
                        out=logits[:], in0=u[:], in1=b2_sb[:],
                        op=mybir.AluOpType.add,
                    )
                    negm = sb.tile([P, 1], f32, tag="negm")
                    nc.vector.reduce_max(
                        out=negm[:], in_=logits[:], axis=mybir.AxisListType.X
                    )
                    nc.vector.tensor_scalar_mul(
                        out=negm[:], in0=negm[:], scalar1=-1.0
                    )
                    expv = sb.tile([P, N_CLASSES], f32, tag="expv")
                    nc.scalar.activation(
                        out=expv[:], in_=logits[:], func=Exp,
                        bias=negm[:, 0:1],
                    )
                    ssum = sb.tile([P, 1], f32, tag="ssum")
                    nc.vector.reduce_sum(
                        out=ssum[:], in_=expv[:], axis=mybir.AxisListType.X
                    )
                    lns = sb.tile([P, 1], f32, tag="lns")
                    nc.scalar.activation(out=lns[:], in_=ssum[:], func=Ln)
                    jj = b % sg
                    if jj == 0:
                        out_acc = sb.tile([P, sg * N_CLASSES], f32,
                                          tag="out_acc", bufs=2)
                    nc.vector.tensor_scalar(
                        out=out_acc[:, jj * N_CLASSES:(jj + 1) * N_CLASSES],
                        in0=logits[:],
                        scalar1=negm[:, 0:1], scalar2=lns[:, 0:1],
                        op0=mybir.AluOpType.add, op1=mybir.AluOpType.subtract,
                    )
                    if jj == sg - 1:
                        b0 = b - sg + 1
                        nc.sync.dma_start(
                            out=out_d[b0 * P:(b + 1) * P, :].rearrange(
                                "(g p) c -> p g c", p=P),
                            in_=out_acc[:].rearrange("p (g c) -> p g c",
                                                     c=N_CLASSES),
                        )
                jp += gsz

    nc.compile()
    return nc


# --------------------------------------------------------------------------
# Host orchestration
# --------------------------------------------------------------------------

def _run(x, edge_index, W1, b1, W2, b2, blocks_per_core):
    from concourse.bass_utils import run_bass_kernel_spmd

    global LAST_RESULT

    x = np.asarray(x, dtype=np.float32)
    W1 = np.asarray(W1, dtype=np.float32)
    b1v = np.asarray(b1, dtype=np.float32).reshape(-1)
    W2 = np.asarray(W2, dtype=np.float32)
    b2v = np.asarray(b2, dtype=np.float32).reshape(-1)

    n_nodes, f_in = x.shape
    hidden = W1.shape[1]
    ncls = W2.shape[1]
    assert hidden == P and ncls == N_CLASSES

    shard = blocks_per_core * P
    n_pad = NCORES * shard

    idxlo, idxhi, segs, dinv_pad, T_lo, T_hi, perm = _preprocess(
        edge_index, n_nodes, blocks_per_core, CH
    )
    T = T_lo + T_hi

    nc = _build_program(f_in, hidden, blocks_per_core, T_lo, T_hi, CH, SG, GB)

    kt = f_in // P
    bf = ml_dtypes.bfloat16

    x_pad = np.zeros((n_pad, f_in), np.float32)
    x_pad[perm] = x
    w1r = np.ascontiguousarray(
        W1.reshape(kt, P, hidden).transpose(1, 0, 2).reshape(P, kt * hidden)
    ).astype(bf)
    w2p = np.zeros((hidden, NCLS_PAD), np.float32)
    w2p[:, :ncls] = W2
    b2t = np.ascontiguousarray(
        np.broadcast_to(b2v[None, :], (P, ncls))
    ).astype(np.float32)
    iotaw = np.ascontiguousarray(
        np.broadcast_to(
            np.tile(np.arange(P, dtype=np.float32), T), (P, T * P)
        )
    ).astype(bf)

    in_maps = []
    for c in range(NCORES):
        xs = x_pad[c * shard:(c + 1) * shard]
        xt4 = np.ascontiguousarray(
            xs.reshape(blocks_per_core, P, kt, P).transpose(3, 0, 2, 1)
            .reshape(P, blocks_per_core * kt * P)
        ).astype(bf)
        dshard = dinv_pad[c * shard:(c + 1) * shard]
        dinvp = np.ascontiguousarray(
            dshard.reshape(blocks_per_core, P).T
        ).astype(np.float32)
        dinvf = np.ascontiguousarray(
            np.broadcast_to(dshard[None, :], (P, shard))
        ).astype(np.float32)
        in_maps.append({
            "xt": xt4,
            "w1": w1r,
            "b1": b1v.reshape(P, 1).copy(),
            "w2": w2p.astype(bf),
            "b2t": b2t,
            "iotaw": iotaw,
            "idxlo": np.ascontiguousarray(idxlo[c]),
            "idxhi": np.ascontiguousarray(idxhi[c]),
            "segs": np.ascontiguousarray(segs[c]).astype(bf),
            "dinvp": dinvp,
            "dinvf": dinvf,
        })

    res = run_bass_kernel_spmd(
        nc, in_maps, core_ids=list(range(NCORES)),
        trace=TRACE, trace_kwargs=dict(TRACE_KWARGS),
    )
    LAST_RESULT = {
        "exec_time_ns": res.exec_time_ns,
        "mean_exec_time_ns": res.mean_exec_time_ns,
        "instructions_and_trace": res.instructions_and_trace,
        "profile_json": res.profile_json,
        "T": T,
        "nc": nc,
        "in_maps": in_maps,
        "perm": perm,
    }
    out = np.concatenate([r["out"] for r in res.results], axis=0)
    return out[perm]


def kernel(x, edge_index, W1, b1, W2, b2):
    n_nodes = np.asarray(x).shape[0]
    blocks_per_core = int(math.ceil(n_nodes / (NCORES * P)))
    return _run(x, edge_index, W1, b1, W2, b2, blocks_per_core)
